# revision 22
# baseline (speedup 1.0000x reference)
"""Multi-head causal attention (dense transformer block) on 8 TRN2 NeuronCores.

Problem: B=2, S=2048, D_MODEL=768, H=12 heads, D_HEAD=64, fp32 I/O.

Sharding: 24 (batch, head) units over 8 cores -> 3 heads x 1 batch per core.
Cores 0-3 handle batch 0 (heads 0-2, 3-5, 6-8, 9-11), cores 4-7 batch 1.
Each core computes its heads' contribution to out[b] = sum_h z_h @ W_O[h];
the host sums the 4 partials per batch and adds b_O.

Per-core dataflow (all matmuls bf16 -> fp32 PSUM):
  - x[b]^T staged in SBUF as chunks [128, 2048]; Q and K projection chains
    staggered at d-chunk granularity so the PE rides the input DMA stream;
    then QK2 + V group 0, V group 1; the last V group overlaps the first
    attention score rounds (nested PSUM pool scopes).
  - Q/K/QK2 outputs in per-512-col tiles so score matmuls depend only on
    the slices they read, not the whole projection.
  - scores TRANSPOSED: sT[k, q] = KT-slice^T x QT (K=64 contraction), so
    softmax normalization is a per-q (free dim) multiply.
  - exp via ScalarE (one instr per [128, 1024] j-pair group), causal mask
    via DVE memset + triangular-mask multiply (bf16 SBUF = DVE 2x/4x modes).
  - zT[h] = sum_j [V_h[j] | 1]^T x PT[j] (ones column makes PSUM row 64 the
    softmax denominator).  DVE copies each accumulator to bf16 SBUF (zsb)
    right away to release PSUM; the denominator row rides along.
  - The reciprocal needs a [128, 12] layout (free-dim cost), reached via a
    DRAM bounce.  All chain DMAs live alone on the Sync queue, and the
    compute steps (reciprocal, normalize muls) are DEFERRED into the next
    q-block's rounds so no engine queue ever blocks on a DMA semaphore.
  - out-proj of block qi is woven into later rounds as 512/256-col pieces
    (1 spare PSUM bank) so the PE never idles; piece DMAs issue from the
    idle Pool queue.  Output DMA'd as bf16; host sums partials in fp32.
  - The LAST q-block's out-proj uses the normalize-commute trick:
    out[q,:] = sum_h r_h[q] * (z_raw_h^T W_O_h)[q,:], where r_h[q] is a
    per-partition scalar — no broadcast hops, so the tail only waits for
    the short half of the reciprocal chain.
"""

import numpy as np
import ml_dtypes
from contextlib import ExitStack

import concourse.bass as bass
import concourse.mybir as mybir
import concourse.tile as tile
from concourse import bacc
from concourse.bass_utils import run_bass_kernel_spmd

BF16 = mybir.dt.bfloat16
F32 = mybir.dt.float32
AF = mybir.ActivationFunctionType
NPBF16 = ml_dtypes.bfloat16

B, S, D, H, DH = 2, 2048, 768, 12, 64
N_CORES = 8
DCH = D // 128          # 6 d_model chunks
NKT = S // 128          # 16 k tiles
QB = 512                # q block width
NQB = S // QB           # 4 q blocks
G = 2                   # k-tiles per exp group

TRACE_ENABLED = False
LAST_EXEC_NS = None
LAST_RESULT = None
_BUILT = None


def build_nc():
    nc = bacc.Bacc("TRN2", target_bir_lowering=False, debug=False)

    xT_d = nc.dram_tensor("xT", [D, S], BF16, kind="ExternalInput")
    wq01_d = nc.dram_tensor("wq01", [D, 128], BF16, kind="ExternalInput")
    wk01_d = nc.dram_tensor("wk01", [D, 128], BF16, kind="ExternalInput")
    wqk2_d = nc.dram_tensor("wqk2", [D, 128], BF16, kind="ExternalInput")
    wv_d = nc.dram_tensor("wv", [D, 195], BF16, kind="ExternalInput")
    wo01_d = nc.dram_tensor("wo01", [128, D], BF16, kind="ExternalInput")
    wo2_d = nc.dram_tensor("wo2", [64, D], BF16, kind="ExternalInput")
    bq01_d = nc.dram_tensor("bq01", [128, 1], F32, kind="ExternalInput")
    bk01_d = nc.dram_tensor("bk01", [128, 1], F32, kind="ExternalInput")
    bqk2_d = nc.dram_tensor("bqk2", [128, 1], F32, kind="ExternalInput")
    bv_d = nc.dram_tensor("bv", [128, 195], F32, kind="ExternalInput")
    out_d = nc.dram_tensor("out_p", [S, D], BF16, kind="ExternalOutput")

    tri_np = np.triu(np.ones((128, 128), np.float32)).astype(NPBF16)
    tri_d = nc.inline_tensor(tri_np, "tri")

    with tile.TileContext(nc) as tc, ExitStack() as ctx:
        persist = ctx.enter_context(tc.tile_pool(name="persist", bufs=1))

        # ---- HAM warm-up: dummy matmuls spanning the input-DMA wait, so
        # the PE clock is ramped when the first projection matmuls issue ----
        with tc.tile_pool(name="warm_ps", bufs=1, space="PSUM") as warm_pool:
            wz = persist.tile([128, 128], BF16, tag="wz")
            nc.vector.memset(wz[:], 0.0)
            wps = warm_pool.tile([128, 128], F32, tag="warm")
            for _ in range(44):
                nc.tensor.matmul(wps[:], wz[:], wz[:], start=True, stop=True)

        # ---- stage inputs in SBUF, issue order matched to consumption ----
        def load_w(dram, cols, tag):
            t = persist.tile([128, DCH * cols], BF16, tag=tag)
            a = dram[:, :]
            src = bass.AP(tensor=a.tensor, offset=a.offset,
                          ap=[[cols, 128], [128 * cols, DCH], [1, cols]])
            nc.sync.dma_start(t[:].rearrange("p (c f) -> p c f", c=DCH), src)
            return t

        def load_small(dram, shape, dt, tag):
            t = persist.tile(shape, dt, tag=tag)
            nc.sync.dma_start(t[:], dram[:, :])
            return t

        wq01 = load_w(wq01_d, 128, "wq01")
        # xt[0] in two column halves (separate tiles) for an earlier start
        xt0a = persist.tile([128, 1024], BF16, tag="xt0a")
        nc.sync.dma_start(xt0a[:], xT_d[0:128, 0:1024])
        xt0b = persist.tile([128, 1024], BF16, tag="xt0b")
        nc.sync.dma_start(xt0b[:], xT_d[0:128, 1024:2048])
        wk01 = load_w(wk01_d, 128, "wk01")
        wqk2 = load_w(wqk2_d, 128, "wqk2")
        bq01 = load_small(bq01_d, [128, 1], F32, "bq01")
        bk01 = load_small(bk01_d, [128, 1], F32, "bk01")
        bqk2 = load_small(bqk2_d, [128, 1], F32, "bqk2")
        xt = [None]
        for d in range(1, DCH):
            t = persist.tile([128, S], BF16, tag=f"xt{d}")
            nc.sync.dma_start(t[:], xT_d[d * 128:(d + 1) * 128, :])
            xt.append(t)
        wv = load_w(wv_d, 195, "wv")
        bv = load_small(bv_d, [128, 195], F32, "bv")
        wo01 = persist.tile([128, D], BF16, tag="wo01")
        nc.sync.dma_start(wo01[:], wo01_d[:, :])
        wo2 = persist.tile([64, D], BF16, tag="wo2")
        nc.sync.dma_start(wo2[:], wo2_d[:, :])
        # head-1 W_O rows at partitions 0-63 (the commute tail contracts
        # each head separately with lhsT on partitions 0-63)
        wo1lo = persist.tile([64, D], BF16, tag="wo1lo")
        nc.sync.dma_start(wo1lo[:], wo01_d[64:128, :])
        tri = load_small(tri_d, [128, 128], BF16, "tri")

        def xslice(d, n0, nw):
            if d == 0:
                t = xt0a if n0 < 1024 else xt0b
                return t[:, n0 % 1024:n0 % 1024 + nw]
            return xt[d][:, n0:n0 + nw]

        # ---- persistent intermediates ----
        # Q/K/QK2 in per-512-col tiles: fine-grained deps let score matmuls
        # start as soon as their slice's bias-add lands.
        def ntiles(tag, parts=128):
            return [persist.tile([parts, 512], BF16, tag=f"{tag}{n}",
                                 name=f"{tag}{n}")
                    for n in range(4)]

        QT01 = ntiles("QT01")
        KT01 = ntiles("KT01")
        QKT2 = ntiles("QKT2")
        KT2lo = ntiles("KT2lo", 64)
        # V in per-4-s-tile tiles: the first PV matmuls depend only on the
        # V slices they read, not the whole projection
        v_sb = [persist.tile([128, 4 * 195], BF16, tag=f"v_sb{i}",
                             name=f"v_sb{i}") for i in range(4)]
        zT01 = persist.tile([128, S], BF16, tag="zT01")
        zT2 = persist.tile([64, S], BF16, tag="zT2")

        def vslice(j, hv):
            base = (j % 4) * 195 + hv * 65
            return v_sb[j // 4][:, base:base + 65]

        def vadd(s_t, ps):
            nc.vector.tensor_add(
                v_sb[s_t // 4][:, (s_t % 4) * 195:(s_t % 4) * 195 + 195],
                ps[:, 0:195], bv[:])

        def kt01(j):
            # KT01 column slice for k-tile j (128 wide)
            return KT01[j // 4][:, (j % 4) * 128:(j % 4) * 128 + 128]

        def kt2(j):
            return KT2lo[j // 4][0:64, (j % 4) * 128:(j % 4) * 128 + 128]

        # ---- QKV projections (phase 1: Q+K staggered, QK2+Vg0, Vg1) ----
        with tc.tile_pool(name="proj_ps", bufs=8, space="PSUM") as proj_pool:
            def chain_tiles(name, n=4):
                return [proj_pool.tile([128, 512], F32, tag="chain",
                                       name=f"{name}{i}") for i in range(n)]

            pq = chain_tiles("q")
            pk = chain_tiles("k")
            for d in range(DCH):
                for w_s, pss in ((wq01, pq), (wk01, pk)):
                    lhsT = w_s[:, d * 128:(d + 1) * 128]
                    for n in range(4):
                        nc.tensor.matmul(pss[n][:], lhsT, xslice(d, n * 512, 512),
                                         start=(d == 0), stop=(d == DCH - 1))
            for pss, bias_s, outs in ((pq, bq01, QT01), (pk, bk01, KT01)):
                for n in range(4):
                    nc.scalar.add(outs[n][:], pss[n][:], bias_s[:])

            # QK2 chain + V group 0 (s_t 0..3) interleaved
            p2 = chain_tiles("qk2")
            vg0 = chain_tiles("v0_")
            for d in range(DCH):
                lhsT = wqk2[:, d * 128:(d + 1) * 128]
                for n in range(4):
                    nc.tensor.matmul(p2[n][:], lhsT, xslice(d, n * 512, 512),
                                     start=(d == 0), stop=(d == DCH - 1))
                for s_t in range(4):
                    nc.tensor.matmul(vg0[s_t][:, 0:195],
                                     xslice(d, s_t * 128, 128),
                                     wv[:, d * 195:(d + 1) * 195],
                                     start=(d == 0), stop=(d == DCH - 1))
            for n in range(4):
                nc.scalar.add(QKT2[n][:], p2[n][:], bqk2[:])
                # head-2 K^T shift to partitions 0-63 (scores h2 needs lhsT
                # and rhs on the same partition range), per 512-col slice
                nc.sync.dma_start(KT2lo[n][:], QKT2[n][64:128, :])
            for s_t in range(4):
                vadd(s_t, vg0[s_t])

            # V group 1 (s_t 4..11), d-major
            vg1 = chain_tiles("v1_", 8)
            for d in range(DCH):
                for i in range(8):
                    s_t = 4 + i
                    nc.tensor.matmul(vg1[i][:, 0:195],
                                     xslice(d, s_t * 128, 128),
                                     wv[:, d * 195:(d + 1) * 195],
                                     start=(d == 0), stop=(d == DCH - 1))
            for i in range(8):
                vadd(4 + i, vg1[i])

        # ---- attention (with V group 2 woven into the first rounds) ----
        # PSUM: sT 2x[128,1024]=4 banks for the whole phase.  The first
        # score rounds coexist with projV2 (4 banks); once that scope
        # closes, zts (3) + op (1) take its banks.
        with tc.tile_pool(name="sT_ps", bufs=2, space="PSUM") as sT_pool, \
             tc.tile_pool(name="pt_sb", bufs=8) as pt_pool, \
             tc.tile_pool(name="zsb_sb", bufs=2) as zsb_pool, \
             tc.tile_pool(name="rb_sb", bufs=2) as rb_pool, \
             tc.tile_pool(name="zs_sb", bufs=2) as zs_pool, \
             tc.tile_pool(name="ob_sb", bufs=4) as ob_pool, \
             tc.tile_pool(name="recip_dr", bufs=2, space="DRAM") as rdr_pool, \
             tc.tile_pool(name="recip_sb", bufs=4) as recip_pool:

            def emit_head_scores(qi, g, hv):
                q0 = qi * QB
                # single-head score round: one [128, 1024] tile for G=2
                # k-tiles of head hv.  Finer rounds -> the 2 sT buffers give
                # two rounds of skew between scores and exp, so neither PE
                # nor ACT ever waits on the other in steady state.
                st = sT_pool.tile([128, G * 512], F32, tag="sT",
                                  name=f"st{g}")
                for jj in range(G):
                    j = g * G + jj
                    osl = slice(jj * 512, (jj + 1) * 512)
                    if hv == 2:
                        nc.tensor.matmul(st[:, osl], kt2(j),
                                         QKT2[qi][0:64, :],
                                         start=True, stop=True)
                    else:
                        p0 = hv * 64
                        nc.tensor.matmul(st[:, osl],
                                         kt01(j)[p0:p0 + 64, :],
                                         QT01[qi][p0:p0 + 64, :],
                                         start=True, stop=True)
                return st

            def exp_mask(qi, g, st, name):
                pt = pt_pool.tile([128, G * 512], BF16, tag="pt", name=name)
                r0 = g * G - 4 * qi
                s0 = r0 * 128 if r0 >= 0 else 0
                nc.scalar.activation(pt[:, s0:G * 512], st[:, s0:G * 512],
                                     AF.Exp)
                for jj in range(G):
                    r = (g * G + jj) - 4 * qi
                    off = jj * 512
                    if r >= 0:
                        if r > 0:
                            nc.vector.memset(pt[:, off:off + r * 128], 0.0)
                        dsl = slice(off + r * 128, off + (r + 1) * 128)
                        nc.vector.tensor_mul(pt[:, dsl], pt[:, dsl], tri[:])
                return pt

            # ---- first two pair-score rounds of q-block 0, woven into
            # V group 2 (s_t 12..15) so the PE rolls from projections into
            # attention without a gap ----
            with tc.tile_pool(name="projv2_ps", bufs=4,
                              space="PSUM") as pv2_pool:
                vg2 = [pv2_pool.tile([128, 512], F32, tag="chain",
                                     name=f"v2_{i}") for i in range(4)]
                warm_rounds = []
                for d in range(DCH):
                    for i in range(4):
                        s_t = 12 + i
                        nc.tensor.matmul(vg2[i][:, 0:195],
                                         xslice(d, s_t * 128, 128),
                                         wv[:, d * 195:(d + 1) * 195],
                                         start=(d == 0), stop=(d == DCH - 1))
                    if d >= 3:
                        hv = d - 3
                        st = emit_head_scores(0, 0, hv)
                        warm_rounds.append(
                            (0, exp_mask(0, 0, st, f"pt{hv}"), hv))
                for i in range(4):
                    vadd(12 + i, vg2[i])

            # ---- main attention machinery ----
            with tc.tile_pool(name="zT_ps", bufs=3, space="PSUM") as zT_pool, \
                 tc.tile_pool(name="op_ps", bufs=1, space="PSUM") as op_pool:

                ob_tiles = {}

                def op_piece(t, p):
                    tsl = slice(t * 128, (t + 1) * 128)
                    c0, cw = (0, 512) if p == 0 else (512, 256)
                    ps = op_pool.tile([128, 512], F32, tag="op",
                                      name=f"op{t}_{p}")
                    nc.tensor.matmul(ps[:, 0:cw], zT01[:, tsl],
                                     wo01[:, c0:c0 + cw], start=True,
                                     stop=False)
                    nc.tensor.matmul(ps[:, 0:cw], zT2[:, tsl],
                                     wo2[:, c0:c0 + cw], start=False,
                                     stop=True)
                    if p == 0:
                        ob = ob_pool.tile([128, D], BF16, tag="ob",
                                          name=f"ob{t}")
                        ob_tiles[t] = ob
                        nc.vector.tensor_copy(ob[:, 0:512], ps[:, 0:512])
                    else:
                        ob = ob_tiles.pop(t)
                        nc.vector.tensor_copy(ob[:, 512:D], ps[:, 0:256])
                        # DMA from the idle Pool queue: never blocks Sync
                        # (reciprocal chain) nor delays DVE
                        nc.gpsimd.dma_start(out_d[tsl, :], ob[:])

                piece_q = []      # pending out-proj pieces
                deferred = []     # (round_idx, fn) chain steps for prev qi

                for qi in range(NQB):
                    q0 = qi * QB
                    J = 4 * qi + 4
                    NG = J // G
                    qsl = slice(q0, q0 + QB)

                    zts = [zT_pool.tile([65, 512], F32, tag="zT",
                                        name=f"zt{i}") for i in range(3)]

                    def pv(g, pt, hv, J=J, zts=zts):
                        for jj in range(G):
                            j = g * G + jj
                            nc.tensor.matmul(
                                zts[hv][:], vslice(j, hv),
                                pt[:, jj * 512:(jj + 1) * 512],
                                start=(j == 0), stop=(j == J - 1))

                    rounds = [(g, hv) for g in range(NG)
                              for hv in range(3)]
                    pending = []

                    # piece filler starts once the previous block's zT is
                    # ready (normalize lands ~round 6); for the last block
                    # hold pieces back so they cover the ACT-bound finish
                    start_ri = max(8, len(rounds) - 12)

                    for ri, (g, hv) in enumerate(rounds):
                        if qi == 0 and g == 0:
                            new_pending = warm_rounds[hv]
                            st = None
                        else:
                            st = emit_head_scores(qi, g, hv)

                        # deferred reciprocal-chain steps of the previous
                        # q-block (their DMA deps have landed by now)
                        while deferred and deferred[0][0] <= ri:
                            deferred.pop(0)[1]()

                        if ri >= start_ri and piece_q:
                            left = len(rounds) - ri - 1
                            budget = 2 if len(piece_q) > left else 1
                            for _ in range(min(budget, len(piece_q))):
                                op_piece(*piece_q.pop(0))

                        for pg, ppt, phv in pending:
                            pv(pg, ppt, phv)
                        if st is None:
                            pending = [new_pending]
                        else:
                            pending = [(g, exp_mask(qi, g, st, f"pt{hv}"),
                                        hv)]
                    for pg, ppt, phv in pending:
                        pv(pg, ppt, phv)

                    # ---- normalize front half (immediately): release PSUM
                    # via DVE casts, then the two reshape DMAs on Sync ----
                    zsb = zsb_pool.tile([65, 3 * 512], BF16, tag="zsb")
                    for h in range(3):
                        nc.vector.tensor_copy(zsb[:, h * 512:(h + 1) * 512],
                                              zts[h][:, :])
                    dr1 = rdr_pool.tile([1, 3 * 512], BF16, tag="dr1")
                    nc.sync.dma_start(dr1[:], zsb[64:65, :])

                    if qi < NQB - 1:
                        # [128, 12] with q-major columns (for broadcast-back)
                        rs = recip_pool.tile([128, 12], BF16, tag="rs")
                        nc.sync.dma_start(
                            rs[:], dr1[:].rearrange("o (p f) -> (o p) f",
                                                    p=128))

                        def back_half(qi=qi, qsl=qsl, zsb=zsb, rs=rs):
                            rr = recip_pool.tile([128, 12], BF16, tag="rr")
                            with nc.allow_low_precision(reason="1/denom bf16"):
                                nc.vector.reciprocal(rr[:], rs[:])
                            dr2 = rdr_pool.tile([1, 3 * 512], BF16, tag="dr2")
                            nc.sync.dma_start(
                                dr2[:].rearrange("o (p f) -> (o p) f", p=128),
                                rr[:])
                            rb = rb_pool.tile([64, 3 * 512], BF16, tag="rb")
                            for h in range(3):
                                nc.sync.dma_start(
                                    rb[:, h * 512:(h + 1) * 512],
                                    dr2[0:1, h * 512:(h + 1) * 512]
                                    .broadcast_to([64, 512]))

                            def muls(qi=qi, qsl=qsl, zsb=zsb, rb=rb):
                                nc.vector.tensor_mul(zT01[0:64, qsl],
                                                     zsb[0:64, 0:512],
                                                     rb[:, 0:512])
                                z1 = zs_pool.tile([64, 512], BF16, tag="z1")
                                nc.vector.tensor_mul(z1[:],
                                                     zsb[0:64, 512:1024],
                                                     rb[:, 512:1024])
                                nc.gpsimd.dma_start(zT01[64:128, qsl], z1[:])
                                nc.vector.tensor_mul(zT2[:, qsl],
                                                     zsb[0:64, 1024:1536],
                                                     rb[:, 1024:1536])
                                for t in range(4 * qi, 4 * qi + 4):
                                    piece_q.append((t, 0))
                                    piece_q.append((t, 1))
                            deferred.append((5, muls))

                        deferred = [(2, back_half)]
                    else:
                        # ---- last block: normalize-commute tail ----
                        # rs_t columns are (h, t)-major: rs_t[p, h*4+ti] =
                        # denom_h[ti*128 + p] -> per-partition recip scalars
                        rs_t = recip_pool.tile([128, 12], BF16, tag="rst")
                        a = dr1[:]
                        src = bass.AP(tensor=a.tensor, offset=a.offset,
                                      ap=[[1, 128], [512, 3], [128, 4]])
                        nc.sync.dma_start(rs_t[:], src)
                        rr_t = recip_pool.tile([128, 12], F32, tag="rrt")
                        nc.vector.reciprocal(rr_t[:], rs_t[:])

                        for ti in range(4):
                            t = 4 * qi + ti
                            tsl = slice(t * 128, (t + 1) * 128)
                            ob = ob_pool.tile([128, D], BF16, tag="ob",
                                              name=f"obt{t}")
                            for h in range(3):
                                ps = sT_pool.tile([128, G * 512], F32,
                                                  tag="sT", name=f"rw{t}_{h}")
                                zraw = zsb[0:64,
                                           h * 512 + ti * 128:
                                           h * 512 + ti * 128 + 128]
                                wo_h = (wo01[0:64, :], wo1lo[:, :],
                                        wo2[:, :])[h]
                                for c0, cw in ((0, 512), (512, 256)):
                                    nc.tensor.matmul(ps[:, c0:c0 + cw], zraw,
                                                     wo_h[:, c0:c0 + cw],
                                                     start=True, stop=True)
                                r_ap = rr_t[:, h * 4 + ti:h * 4 + ti + 1]
                                if h == 0:
                                    nc.scalar.activation(ob[:], ps[:, 0:D],
                                                         AF.Copy,
                                                         scale=r_ap)
                                else:
                                    nc.vector.scalar_tensor_tensor(
                                        ob[:], ps[:, 0:D], r_ap, ob[:],
                                        op0=mybir.AluOpType.mult,
                                        op1=mybir.AluOpType.add)
                            nc.gpsimd.dma_start(out_d[tsl, :], ob[:])

                # drain any pieces not yet emitted (shouldn't happen, but
                # keep the output complete if budgets change)
                while piece_q:
                    op_piece(*piece_q.pop(0))

    nc.compile()
    return nc


def _get_nc():
    global _BUILT
    if _BUILT is None:
        _BUILT = build_nc()
    return _BUILT


def make_in_maps(inputs):
    x = np.asarray(inputs["normalized_resid_pre"], dtype=np.float32)
    W_Q = np.asarray(inputs["W_Q"], dtype=np.float32)
    W_K = np.asarray(inputs["W_K"], dtype=np.float32)
    W_V = np.asarray(inputs["W_V"], dtype=np.float32)
    W_O = np.asarray(inputs["W_O"], dtype=np.float32)
    b_Q = np.asarray(inputs["b_Q"], dtype=np.float32)
    b_K = np.asarray(inputs["b_K"], dtype=np.float32)
    b_V = np.asarray(inputs["b_V"], dtype=np.float32)
    sc = 1.0 / np.sqrt(np.float32(DH))

    in_maps = []
    for c in range(N_CORES):
        b = c // 4
        h = (c % 4) * 3
        hs = [h, h + 1, h + 2]
        m = {
            "xT": np.ascontiguousarray(x[b].T).astype(NPBF16),
            "wq01": np.concatenate([W_Q[hs[0]] * sc, W_Q[hs[1]] * sc],
                                   axis=1).astype(NPBF16),
            "wk01": np.concatenate([W_K[hs[0]], W_K[hs[1]]], axis=1).astype(NPBF16),
            "wqk2": np.concatenate([W_Q[hs[2]] * sc, W_K[hs[2]]],
                                   axis=1).astype(NPBF16),
            "wv": np.concatenate(
                sum(([W_V[hh], np.zeros((D, 1), np.float32)] for hh in hs), []),
                axis=1).astype(NPBF16),
            "wo01": np.concatenate([W_O[hs[0]], W_O[hs[1]]], axis=0).astype(NPBF16),
            "wo2": W_O[hs[2]].astype(NPBF16),
            "bq01": (np.concatenate([b_Q[hs[0]], b_Q[hs[1]]]) * sc)[:, None]
                    .astype(np.float32),
            "bk01": np.concatenate([b_K[hs[0]], b_K[hs[1]]])[:, None]
                    .astype(np.float32),
            "bqk2": np.concatenate([b_Q[hs[2]] * sc, b_K[hs[2]]])[:, None]
                    .astype(np.float32),
            "bv": np.ascontiguousarray(np.broadcast_to(
                np.concatenate(
                    sum(([b_V[hh], np.ones(1, np.float32)] for hh in hs), [])),
                (128, 195))).astype(np.float32),
        }
        in_maps.append(m)
    return in_maps


def kernel(**inputs):
    global LAST_EXEC_NS, LAST_RESULT
    nc = _get_nc()
    in_maps = make_in_maps(inputs)
    b_O = np.asarray(inputs["b_O"], dtype=np.float32)

    res = run_bass_kernel_spmd(nc, in_maps, core_ids=list(range(N_CORES)),
                               trace=TRACE_ENABLED)
    LAST_EXEC_NS = res.exec_time_ns
    LAST_RESULT = res
    parts = [np.asarray(r["out_p"], dtype=np.float32) for r in res.results]
    out0 = parts[0] + parts[1] + parts[2] + parts[3]
    out1 = parts[4] + parts[5] + parts[6] + parts[7]
    out = np.stack([out0, out1]) + b_O
    return out.astype(np.float32)
